# revision 14
# baseline (speedup 1.0000x reference)
"""AxialShift block on 8 TRN2 NeuronCores (Bass/Tile, SPMD).

Computation (see the nn.Module reference):
    h   = gelu(groupnorm1(conv1x1(x, w1, b1), g1, bt1))
    x_a = axial_shift(pad(h), axis=a) for a in D,H,W  (3 channel chunks
          shifted by -1/0/+1 along the axis, zero boundary)
    y   = sum_a gelu(conv1x1(x_a, w2a, b2a))
    out = conv1x1(groupnorm1(y, g2, bt2), w3, b3)

Sharding: core k -> (b = k//4, d-slices [8k%32, +8)). Halo of 1 D-slice is
recomputed locally (host pre-pads x with zeros at sample edges). GroupNorm
stats are all-reduced across the 4 cores of each sample as 2 scalars.

Schedule (all matmuls in [128, 512]-free accumulation groups packed two to
a [128,1024] 2-bank PSUM tile):
  phase A: conv1 for interior d-slices 1..8 (stats slices) -> gn1 partial
           stats + AllReduce trigger; halo slices 0,9 computed while the
           collective is in flight. PSUM->h drains on ScalarE (Copy),
           bn_stats on VectorE.
  phase B: per output d-slice (dout 1..8): gelu(h) for the slices as they
           become needed; H- and W-axis convs first (only need slice dout),
           D-axis conv last (needs dout+-1). gelu straight from PSUM in
           [128,1024] chunks with the conv bias folded into the activation.
           y accumulates in SBUF bf16 (no spill). conv3 for dout-2
           interleaves to keep PE dense.
  phase C: gn2 stats aggregate -> AllReduce, hidden under conv3 of the
           last two d-slices; per-channel epilogue on VectorE in place on
           the staged conv3 output (aliases y), out DMA'd as bf16 and
           widened to f32 on host.
"""

import numpy as np

DIM = 384
R = 32
B = 2
EPS = 1e-5

DSH = 8                  # own D-slices per core
DTOT = DSH + 2           # + halo
SLICE = 33 * 33          # padded 32x32 plane with shared zero row/col
HBUF = DTOT * SLICE + 1  # +1 head zero element
TOK_OUT = DSH * R * R    # 8192
NTOT = float(DIM * R * R * R)  # elements per sample for groupnorm

# rows of the packed per-channel vector input
VB1, VG1, VBT1, VB21, VB22, VB23, VAV, VBV = range(8)

_compiled = None


def _build(gelu_func=None):
    import concourse.bass as bass
    import concourse.bacc as bacc
    import concourse.tile as tile
    from concourse import mybir

    f32 = mybir.dt.float32
    bf16 = mybir.dt.bfloat16
    AF = mybir.ActivationFunctionType
    OP = mybir.AluOpType
    GELU = gelu_func if gelu_func is not None else AF.Gelu

    nc = bacc.Bacc("TRN2", target_bir_lowering=False, debug=False, num_devices=8)

    xs = nc.dram_tensor("xs", [DIM, DTOT * 1024], bf16, kind="ExternalInput")
    w1t = nc.dram_tensor("w1t", [DIM, DIM], bf16, kind="ExternalInput")
    w2lt = nc.dram_tensor("w2lt", [DIM, DIM], bf16, kind="ExternalInput")
    w2tt = nc.dram_tensor("w2tt", [DIM, DIM], bf16, kind="ExternalInput")
    w2ht = nc.dram_tensor("w2ht", [DIM, DIM], bf16, kind="ExternalInput")
    w3t = nc.dram_tensor("w3t", [DIM, DIM], bf16, kind="ExternalInput")
    vecs = nc.dram_tensor("vecs", [8, DIM], f32, kind="ExternalInput")
    hm = nc.dram_tensor("hm", [2], f32, kind="ExternalInput")
    zpad = nc.dram_tensor("zpad", [330], bf16, kind="ExternalInput")
    out_d = nc.dram_tensor("out", [DIM, TOK_OUT], bf16, kind="ExternalOutput")

    cc1_in = nc.dram_tensor("cc1_in", [2], f32)
    cc1_out = nc.dram_tensor("cc1_out", [2], f32)
    cc2_in = nc.dram_tensor("cc2_in", [2], f32)
    cc2_out = nc.dram_tensor("cc2_out", [2], f32)
    GROUPS = [[0, 1, 2, 3], [4, 5, 6, 7]]

    with tile.TileContext(nc) as tc:
        with (
            tc.tile_pool(name="const", bufs=1) as cpool,
            tc.tile_pool(name="hpool", bufs=1) as hpool,
            tc.tile_pool(name="ypool", bufs=1) as ypool,
            tc.tile_pool(name="stat", bufs=1) as spool,
            tc.tile_pool(name="vecp", bufs=1) as vpool,
            tc.tile_pool(name="xin", bufs=3) as xpool,
            tc.tile_pool(name="tmp", bufs=3) as tpool,
            tc.tile_pool(name="ps", bufs=3, space="PSUM") as pspool,
            tc.tile_pool(name="psr", bufs=1, space="PSUM") as psrpool,
        ):
            # ---------- constants ----------
            w1sb = [cpool.tile([128, DIM], bf16, tag=f"w1_{j}", name=f"w1_{j}") for j in range(3)]
            w2lsb = [cpool.tile([128, DIM], bf16, tag=f"w2l_{j}", name=f"w2l_{j}") for j in range(3)]
            w2tsb = [cpool.tile([128, DIM], bf16, tag=f"w2t_{j}", name=f"w2t_{j}") for j in range(3)]
            w2hsb = [cpool.tile([128, DIM], bf16, tag=f"w2h_{j}", name=f"w2h_{j}") for j in range(3)]
            w3sb = [cpool.tile([128, DIM], bf16, tag=f"w3_{j}", name=f"w3_{j}") for j in range(3)]
            # w1 on the SP queue (feeds the first matmuls); the phase-B/C
            # weights go on the idle GpSimd queue so they don't delay the
            # x-tile DMA stream behind 12 serialized dispatches
            for j in range(3):
                sl = slice(j * 128, (j + 1) * 128)
                nc.sync.dma_start(out=w1sb[j][:], in_=w1t[sl, :])
            for j in range(3):
                sl = slice(j * 128, (j + 1) * 128)
                nc.gpsimd.dma_start(out=w2lsb[j][:], in_=w2lt[sl, :])
                nc.gpsimd.dma_start(out=w2hsb[j][:], in_=w2ht[sl, :])
                nc.gpsimd.dma_start(out=w2tsb[j][:], in_=w2tt[sl, :])
                nc.gpsimd.dma_start(out=w3sb[j][:], in_=w3t[sl, :])

            vt = cpool.tile([128, 8, 3], f32, tag="vecs", name="vecs")
            nc.gpsimd.dma_start(
                out=vt[:],
                in_=bass.AP(tensor=vecs.ap().tensor, offset=0,
                            ap=[[1, 128], [DIM, 8], [128, 3]]),
            )

            def vec(r, m):
                return vt[:, r, m:m + 1]

            hmb = cpool.tile([128, 2], f32, tag="hm", name="hm")
            nc.gpsimd.dma_start(
                out=hmb[:],
                in_=bass.AP(tensor=hm.ap().tensor, offset=0, ap=[[0, 128], [1, 2]]),
            )
            eps_t = cpool.tile([128, 1], f32, tag="eps", name="eps")
            nc.vector.memset(eps_t[:], EPS)
            ones = cpool.tile([128, 1], f32, tag="ones", name="ones")
            nc.vector.memset(ones[:], 1.0)
            # pre-warm the sqrt ACT table set so the gn1-critical-path Sqrt
            # doesn't pay the table load
            scr1 = cpool.tile([128, 1], f32, tag="scr1", name="scr1")
            nc.scalar.activation(out=scr1[:], in_=eps_t[:], func=AF.Sqrt)

            hb = [hpool.tile([128, HBUF], bf16, tag=f"hb{m}", name=f"hb{m}") for m in range(3)]
            yb = [ypool.tile([128, TOK_OUT], bf16, tag=f"yb{m}", name=f"yb{m}") for m in range(3)]
            zsb = cpool.tile([128, 330], bf16, tag="zsb", name="zsb")
            nc.gpsimd.dma_start(
                out=zsb[:],
                in_=bass.AP(tensor=zpad.ap().tensor, offset=0,
                            ap=[[0, 128], [1, 330]]),
            )
            zv = zsb[:].rearrange("p (a b) -> p a b", a=DTOT)
            for m in range(3):
                nc.scalar.activation(out=hb[m][:, 0:1], in_=zsb[:, 0:1], func=AF.Copy)
                hv = hb[m][:, 1:].rearrange("p (d h w) -> p d h w", d=DTOT, h=33)
                nc.scalar.activation(out=hv[:, :, 32, :], in_=zv, func=AF.Copy)
                nc.scalar.activation(out=hv[:, :, :, 32], in_=zv, func=AF.Copy)

            st1 = [spool.tile([128, 16, 6], f32, tag=f"st1_{m}", name=f"st1_{m}") for m in range(3)]
            st2 = [spool.tile([128, 16, 6], f32, tag=f"st2_{m}", name=f"st2_{m}") for m in range(3)]

            def vtile(tag):
                return vpool.tile([128, 1], f32, tag=tag, name=tag)

            def vtile2(tag):
                return vpool.tile([128, 2], f32, tag=tag, name=tag)

            # ---------- phase A: conv1 into padded h (pre-norm) ----------
            def conv1_slice(d):
                xt = [xpool.tile([128, 1024], bf16, tag=f"xt{j}", name=f"xt{j}") for j in range(3)]
                for j in range(3):
                    nc.sync.dma_start(
                        out=xt[j][:],
                        in_=xs[j * 128:(j + 1) * 128, d * 1024:(d + 1) * 1024],
                    )
                for m in range(3):
                    ps = pspool.tile([128, 1024], f32, tag="ps", name="ps")
                    for j in range(3):
                        for half in range(2):
                            nc.tensor.matmul(
                                ps[:, half * 512:(half + 1) * 512],
                                w1sb[j][:, m * 128:(m + 1) * 128],
                                xt[j][:, half * 512:(half + 1) * 512],
                                start=(j == 0), stop=(j == 2),
                            )
                    hv = hb[m][:, 1 + d * SLICE:1 + (d + 1) * SLICE].rearrange(
                        "p (h w) -> p h w", h=33)
                    nc.scalar.activation(
                        out=hv[:, 0:32, 0:32],
                        in_=ps[:].rearrange("p (h w) -> p h w", h=32),
                        func=AF.Copy,
                    )
                    if 1 <= d <= 8:
                        for half in range(2):
                            nc.vector.bn_stats(
                                out=st1[m][:, (d - 1) * 2 + half, :],
                                in_=ps[:, half * 512:(half + 1) * 512])

            for d in range(1, 9):
                conv1_slice(d)

            # ---------- gn1 stats aggregate + collective (halo overlaps) ----
            sbq1 = [vtile2(f"sbq1_{m}") for m in range(3)]
            for m in range(3):
                mv = vtile2(f"mv1_{m}")
                nc.vector.bn_aggr(out=mv[:], in_=st1[m][:])
                # col0: sum with bias = 8192*(mean + b1)
                nc.vector.tensor_scalar(
                    out=sbq1[m][:, 0:1], in0=mv[:, 0:1],
                    scalar1=vec(VB1, m), scalar2=float(TOK_OUT),
                    op0=OP.add, op1=OP.mult,
                )
                # col1: sumsq with bias = 8192*var + sum^2/8192
                tsq = vtile(f"tsq1_{m}")
                nc.vector.tensor_mul(tsq[:], sbq1[m][:, 0:1], sbq1[m][:, 0:1])
                tv8 = vtile(f"tv81_{m}")
                nc.vector.tensor_scalar_mul(tv8[:], in0=mv[:, 1:2],
                                            scalar1=float(TOK_OUT))
                nc.vector.tensor_scalar(
                    out=sbq1[m][:, 1:2], in0=tsq[:],
                    scalar1=1.0 / TOK_OUT, scalar2=tv8[:],
                    op0=OP.mult, op1=OP.add,
                )
            psr = psrpool.tile([1, 2], f32, tag="psr1", name="psr1")
            for m in range(3):
                nc.tensor.matmul(psr[:], ones[:], sbq1[m][:],
                                 start=(m == 0), stop=(m == 2))
            prs = vpool.tile([1, 2], f32, tag="prs1", name="prs1")
            nc.vector.tensor_copy(out=prs[:], in_=psr[:])
            nc.sync.dma_start(out=cc1_in[:], in_=prs[:])
            nc.gpsimd.collective_compute(
                "AllReduce", OP.add, replica_groups=GROUPS,
                ins=[cc1_in.ap().opt()], outs=[cc1_out.ap().opt()],
            )

            # halo conv1 slices run while the collective is in flight
            conv1_slice(0)
            conv1_slice(9)

            gstat1 = vtile2("gstat1")
            nc.sync.dma_start(
                out=gstat1[:],
                in_=bass.AP(tensor=cc1_out.ap().tensor, offset=0,
                            ap=[[0, 128], [1, 2]]),
            )
            mu1 = vtile("mu1")
            nc.vector.tensor_scalar_mul(mu1[:], in0=gstat1[:, 0:1], scalar1=1.0 / NTOT)
            m21 = vtile("m21")
            nc.vector.tensor_scalar_mul(m21[:], in0=gstat1[:, 1:2], scalar1=1.0 / NTOT)
            var1 = vtile("var1")
            nc.vector.tensor_mul(var1[:], mu1[:], mu1[:])
            nc.vector.tensor_sub(var1[:], m21[:], var1[:])
            sd1 = vtile("sd1")
            nc.scalar.activation(out=sd1[:], in_=var1[:], func=AF.Sqrt,
                                 bias=eps_t[:], scale=1.0)
            rstd1 = vtile("rstd1")
            nc.vector.reciprocal(rstd1[:], sd1[:])
            # all three m chunks at once on [128,3]-wide tiles
            def vtile3(tag):
                return vpool.tile([128, 3], f32, tag=tag, name=tag)
            svv, tvv = vtile3("svv"), vtile3("tvv")
            nc.vector.tensor_scalar_mul(svv[:], in0=vt[:, VG1, :], scalar1=rstd1[:])
            nc.vector.tensor_scalar_sub(out=tvv[:], in0=vt[:, VB1, :],
                                        scalar1=mu1[:])
            nc.vector.tensor_mul(tvv[:], tvv[:], svv[:])
            nc.vector.tensor_add(tvv[:], tvv[:], vt[:, VBT1, :])
            svvlo, tvvlo = vtile3("svvlo"), vtile3("tvvlo")
            svvhi, tvvhi = vtile3("svvhi"), vtile3("tvvhi")
            nc.vector.tensor_scalar_mul(svvlo[:], in0=svv[:], scalar1=hmb[:, 0:1])
            nc.vector.tensor_scalar_mul(tvvlo[:], in0=tvv[:], scalar1=hmb[:, 0:1])
            nc.vector.tensor_scalar_mul(svvhi[:], in0=svv[:], scalar1=hmb[:, 1:2])
            nc.vector.tensor_scalar_mul(tvvhi[:], in0=tvv[:], scalar1=hmb[:, 1:2])
            sv = [svv[:, m:m + 1] for m in range(3)]
            tv = [tvv[:, m:m + 1] for m in range(3)]
            svlo = [svvlo[:, m:m + 1] for m in range(3)]
            tvlo = [tvvlo[:, m:m + 1] for m in range(3)]
            svhi = [svvhi[:, m:m + 1] for m in range(3)]
            tvhi = [tvvhi[:, m:m + 1] for m in range(3)]

            # ---------- phase B: gelu(h) + shifted convs + conv3 ----------
            gelu_done = set()

            def gelu_slice(d):
                if d in gelu_done:
                    return
                gelu_done.add(d)
                for m in range(3):
                    hv = hb[m][:, 1 + d * SLICE:1 + (d + 1) * SLICE].rearrange(
                        "p (h w) -> p h w", h=33)
                    ap = hv[:, 0:32, 0:32]
                    if d == 0:
                        s_m, t_m = svlo[m], tvlo[m]
                    elif d == DTOT - 1:
                        s_m, t_m = svhi[m], tvhi[m]
                    else:
                        s_m, t_m = sv[m], tv[m]
                    nc.scalar.activation(out=ap, in_=ap, func=GELU,
                                         bias=t_m, scale=s_m)

            # H-axis (stride 33) and W-axis (stride 1) only need slice dout;
            # D-axis (stride SLICE) needs dout+-1 and runs in a second pass.
            AX_H = (w2lsb, 33, VB21)
            AX_W = (w2hsb, 1, VB23)
            AX_D = (w2tsb, SLICE, VB22)

            def conv2_axis(dout, axis, first):
                wsb, stp, bvrow = axis
                ysl = [yb[m][:, (dout - 1) * 1024:dout * 1024] for m in range(3)]
                for m in range(3):
                    ps = pspool.tile([128, 1024], f32, tag="ps", name="ps")
                    for j in range(3):
                        off0 = 1 + dout * SLICE - (j - 1) * stp
                        for half in range(2):
                            off = off0 + half * 16 * 33
                            rhs = hb[j][:, off:off + 16 * 33].rearrange(
                                "p (h w) -> p h w", h=16)[:, :, 0:32]
                            nc.tensor.matmul(
                                ps[:, half * 512:(half + 1) * 512],
                                wsb[j][:, m * 128:(m + 1) * 128],
                                rhs,
                                start=(j == 0), stop=(j == 2),
                            )
                    if first:
                        nc.scalar.activation(out=ysl[m], in_=ps[:],
                                             func=GELU, bias=vec(bvrow, m))
                    else:
                        tmp = tpool.tile([128, 1024], bf16, tag="tmp", name="tmp")
                        nc.scalar.activation(out=tmp[:], in_=ps[:],
                                             func=GELU, bias=vec(bvrow, m))
                        nc.vector.tensor_add(ysl[m], ysl[m], tmp[:])
                        if axis is AX_D:
                            for half in range(2):
                                nc.vector.bn_stats(
                                    out=st2[m][:, (dout - 1) * 2 + half, :],
                                    in_=ysl[m][:, half * 512:(half + 1) * 512])

            # conv3 output stages in place over y (slice dout is dead once
            # all three j-chunks of it have been consumed by conv3)
            def conv3_dout(dout):
                # all 18 matmuls read y[0..2] before any drain overwrites it
                # (the stage aliases y), so drain only after the whole slice
                lo = (dout - 1) * 1024
                pss = []
                for m in range(3):
                    ps = pspool.tile([128, 1024], f32, tag="ps", name="ps")
                    for j in range(3):
                        for half in range(2):
                            sl = slice(lo + half * 512, lo + (half + 1) * 512)
                            nc.tensor.matmul(
                                ps[:, half * 512:(half + 1) * 512],
                                w3sb[j][:, m * 128:(m + 1) * 128],
                                yb[j][:, sl],
                                start=(j == 0), stop=(j == 2),
                            )
                    pss.append(ps)
                for m in range(3):
                    nc.vector.tensor_copy(out=yb[m][:, lo:lo + 1024], in_=pss[m][:])

            # pass 1: H+W convs (gelu of one slice unblocks each dout)
            for dout in range(1, 9):
                gelu_slice(dout)
                conv2_axis(dout, AX_H, first=True)
                conv2_axis(dout, AX_W, first=False)
                if dout == 2:
                    gelu_slice(0)
                elif dout == 4:
                    gelu_slice(9)
            # pass 2: D convs (finalize y) with conv3 lagging two slices
            for dout in range(1, 9):
                conv2_axis(dout, AX_D, first=False)
                if dout >= 3:
                    conv3_dout(dout - 2)

            # ---------- gn2 stats aggregate + collective ----------
            sbq2 = [vtile2(f"sbq2_{m}") for m in range(3)]
            for m in range(3):
                mv = vtile2(f"mv2_{m}")
                nc.vector.bn_aggr(out=mv[:], in_=st2[m][:])
                nc.vector.tensor_scalar_mul(sbq2[m][:, 0:1], in0=mv[:, 0:1],
                                            scalar1=float(TOK_OUT))
                tsq = vtile(f"tsq2_{m}")
                nc.vector.tensor_mul(tsq[:], mv[:, 0:1], mv[:, 0:1])
                nc.vector.tensor_add(tsq[:], tsq[:], mv[:, 1:2])
                nc.vector.tensor_scalar_mul(sbq2[m][:, 1:2], in0=tsq[:],
                                            scalar1=float(TOK_OUT))
            psr2 = psrpool.tile([1, 2], f32, tag="psr2", name="psr2")
            for m in range(3):
                nc.tensor.matmul(psr2[:], ones[:], sbq2[m][:],
                                 start=(m == 0), stop=(m == 2))
            prs2 = vpool.tile([1, 2], f32, tag="prs2", name="prs2")
            nc.vector.tensor_copy(out=prs2[:], in_=psr2[:])
            nc.sync.dma_start(out=cc2_in[:], in_=prs2[:])
            nc.gpsimd.collective_compute(
                "AllReduce", OP.add, replica_groups=GROUPS,
                ins=[cc2_in.ap().opt()], outs=[cc2_out.ap().opt()],
            )

            # last two conv3 slices run while the collective is in flight
            conv3_dout(7)
            conv3_dout(8)

            gstat2 = vtile2("gstat2")
            nc.sync.dma_start(
                out=gstat2[:],
                in_=bass.AP(tensor=cc2_out.ap().tensor, offset=0,
                            ap=[[0, 128], [1, 2]]),
            )
            mu2 = vtile("mu2")
            nc.vector.tensor_scalar_mul(mu2[:], in0=gstat2[:, 0:1], scalar1=1.0 / NTOT)
            m22 = vtile("m22")
            nc.vector.tensor_scalar_mul(m22[:], in0=gstat2[:, 1:2], scalar1=1.0 / NTOT)
            var2 = vtile("var2")
            nc.vector.tensor_mul(var2[:], mu2[:], mu2[:])
            nc.vector.tensor_sub(var2[:], m22[:], var2[:])
            sd2 = vtile("sd2")
            nc.scalar.activation(out=sd2[:], in_=var2[:], func=AF.Sqrt,
                                 bias=eps_t[:], scale=1.0)
            rstd2 = vtile("rstd2")
            nc.vector.reciprocal(rstd2[:], sd2[:])
            p2 = vtile("p2")
            nc.vector.tensor_mul(p2[:], mu2[:], rstd2[:])
            cst = []
            for m in range(3):
                c_m = vtile(f"cst_{m}")
                nc.vector.tensor_mul(c_m[:], vec(VAV, m), p2[:])
                nc.vector.tensor_sub(c_m[:], vec(VBV, m), c_m[:])
                cst.append(c_m)

            # ---------- phase C: per-channel epilogue + out DMA ----------
            for q in range(4):
                for m in range(3):
                    chunk = yb[m][:, q * 2048:(q + 1) * 2048]
                    nc.vector.tensor_scalar(
                        out=chunk, in0=chunk, scalar1=rstd2[:], scalar2=cst[m][:],
                        op0=OP.mult, op1=OP.add,
                    )
                    nc.sync.dma_start(
                        out=out_d[m * 128:(m + 1) * 128, q * 2048:(q + 1) * 2048],
                        in_=chunk,
                    )

    nc.compile()
    return nc


def _prepare_in_maps(inputs):
    import ml_dtypes

    f = np.float32
    x = np.asarray(inputs["x"], f)
    w1 = np.asarray(inputs["w1"], f)
    b1 = np.asarray(inputs["b1"], f)
    g1 = np.asarray(inputs["g1"], f)
    bt1 = np.asarray(inputs["bt1"], f)
    w21 = np.asarray(inputs["w21"], f)
    b21 = np.asarray(inputs["b21"], f)
    w22 = np.asarray(inputs["w22"], f)
    b22 = np.asarray(inputs["b22"], f)
    w23 = np.asarray(inputs["w23"], f)
    b23 = np.asarray(inputs["b23"], f)
    g2 = np.asarray(inputs["g2"], f)
    bt2 = np.asarray(inputs["bt2"], f)
    w3 = np.asarray(inputs["w3"], f)
    b3 = np.asarray(inputs["b3"], f)

    w1t = np.ascontiguousarray(w1.T).astype(ml_dtypes.bfloat16)
    # x_lr shifts along H and uses w21; x_td along D uses w22; x_hd along W, w23
    w2lt = np.ascontiguousarray(w21.T).astype(ml_dtypes.bfloat16)
    w2tt = np.ascontiguousarray(w22.T).astype(ml_dtypes.bfloat16)
    w2ht = np.ascontiguousarray(w23.T).astype(ml_dtypes.bfloat16)
    w3g = w3 * g2[None, :]
    w3t = np.ascontiguousarray(w3g.T).astype(ml_dtypes.bfloat16)
    avec = w3 @ g2
    bvec = b3 + w3 @ bt2
    vecs = np.ascontiguousarray(
        np.stack([b1, g1, bt1, b21, b22, b23, avec, bvec]).astype(f))

    in_maps = []
    for core in range(8):
        b, d0 = core // 4, (core % 4) * DSH
        xsh = np.zeros((DIM, DTOT, R, R), f)
        lo, hi = d0 - 1, d0 + DSH + 1
        s0, s1 = max(lo, 0), min(hi, R)
        xsh[:, s0 - lo:s0 - lo + (s1 - s0)] = x[b, :, s0:s1]
        hmv = np.array([0.0 if d0 == 0 else 1.0,
                        0.0 if d0 + DSH == R else 1.0], f)
        in_maps.append(dict(
            xs=np.ascontiguousarray(xsh.reshape(DIM, DTOT * 1024)).astype(
                ml_dtypes.bfloat16),
            zpad=np.zeros(330, ml_dtypes.bfloat16),
            w1t=w1t, w2lt=w2lt, w2tt=w2tt, w2ht=w2ht, w3t=w3t,
            vecs=vecs, hm=hmv,
        ))
    return in_maps


def _gather(results):
    out = np.empty((B, DIM, R, R, R), np.float32)
    for core in range(8):
        b, d0 = core // 4, (core % 4) * DSH
        out[b, :, d0:d0 + DSH] = (
            results[core]["out"].astype(np.float32).reshape(DIM, DSH, R, R))
    return out


def _run(inputs, trace=False, tmpdir=None):
    global _compiled
    if _compiled is None:
        _compiled = _build()
    from concourse import bass_utils

    in_maps = _prepare_in_maps(inputs)
    res = bass_utils.run_bass_kernel_spmd(
        _compiled, in_maps, core_ids=list(range(8)), trace=trace, tmpdir=tmpdir)
    return _gather(res.results), res


def kernel(**inputs) -> np.ndarray:
    out, _ = _run(inputs)
    return out
